# revision 1
# baseline (speedup 1.0000x reference)
"""Trainium2 Bass kernel for nn_ATVP_router_UNI (moe_routing).

Sharding: output dim D=1536 sharded over 8 cores (192 each). Activations
(x_enc, x_ib) are broadcast in transposed layout; the router MLP is
replicated on every core. The expert mean over e commutes with the linear
projection, so each core streams its W_proj slice once, reduces over e on
DVE, and runs 1/7 of the naive matmul FLOPs in fp32r (all streams over
HWDGE; fp32r rounding happens in ACT copies / DVE outputs since the PE
requires pre-rounded fp32r operands). The softmax denominator and the 1/7
group-mean scales cancel inside the final l2 normalization, so they are
folded away. The only cross-core coupling is the two l2 norms (uni-branch
and final): a 4KB AllReduce of the uni sum-of-squares fired early (fully
hidden under streaming) and an 8KB AllReduce of the out-norm partials on
the tail.

kernel(**inputs) takes the full unsharded inputs and returns the full
[1024, 1536] output. Host-side prep does layout only (transpose / slice /
constant staging) -- all arithmetic runs on device.
"""

import numpy as np

import concourse.bass as bass
import concourse.tile as tile
import concourse.mybir as mybir
from concourse import bacc
from concourse.bass_utils import run_bass_kernel_spmd

f32 = mybir.dt.float32
f32r = mybir.dt.float32r
AX = mybir.AxisListType
ALU = mybir.AluOpType
ACTF = mybir.ActivationFunctionType

NCORES = 8
B, N, G, K, D = 1024, 10, 7, 1024, 1536
DS = D // NCORES          # 192 output dims per core
KT = K // 128             # 8 k-tiles
BSL = 512                 # free-dim slice for matmuls
NBS = B // BSL            # 2
ROUTER_GS = (0, 4, 8)     # TEXT_PRED groups, also the router input order
LATE_GS = (1, 2, 3, 5, 6, 7, 9)
CHUNKS = ((0, 128), (128, 64))  # (d-offset, size) chunks of DS=192
EPS_BN = 1e-5
EPS_NORM = 1e-12

LAST_RESULTS = None
_NC_CACHE = {}


def _emit(nc, tc):
    xT_d = nc.dram_tensor("xT", [N, K, B], f32, kind="ExternalInput").ap()
    xibT_d = nc.dram_tensor("xibT", [K, B], f32, kind="ExternalInput").ap()
    w_d = nc.dram_tensor("w", [N, K, DS, G], f32, kind="ExternalInput").ap()
    wib_d = nc.dram_tensor("wib", [K, DS, G], f32, kind="ExternalInput").ap()
    xuT_d = nc.dram_tensor("xuT", [DS, B], f32, kind="ExternalInput").ap()
    bp_d = nc.dram_tensor("bp", [G, N, DS], f32, kind="ExternalInput").ap()
    bib_d = nc.dram_tensor("bib", [G, DS], f32, kind="ExternalInput").ap()
    rw1_d = nc.dram_tensor("rw1", [3 * K, BSL], f32, kind="ExternalInput").ap()
    rw2_d = nc.dram_tensor("rw2", [128, 4, 100], f32, kind="ExternalInput").ap()
    rw3_d = nc.dram_tensor("rw3", [100, 11], f32, kind="ExternalInput").ap()
    rb3_d = nc.dram_tensor("rb3", [11, 1], f32, kind="ExternalInput").ap()
    sel_d = nc.dram_tensor("sel", [11, 11, 128], f32, kind="ExternalInput").ap()
    ones_d = nc.dram_tensor("onesd", [128, BSL], f32, kind="ExternalInput").ap()
    twos_d = nc.dram_tensor("twosd", [128, 1], f32, kind="ExternalInput").ap()
    outT_d = nc.dram_tensor("outT", [DS, B], f32, kind="ExternalOutput").ap()

    pools = {}

    def pool(name, bufs, space="SBUF"):
        cm = tc.tile_pool(name=name, bufs=bufs, space=space)
        pools[name] = cm
        return cm.__enter__()

    cp = pool("const", 1)       # persistent constants / buffers
    xtp = pool("xt", 8)         # [128, B] f32r activation k-tiles (shared ring)
    xsp = pool("xts", 4)        # [128, B] f32 staging for HWDGE loads
    wtp = pool("wt", 3)         # [128, DS, G] weight k-tiles
    wsp = pool("ws", 3)         # [128, DS] e-reduced weights
    rwp = pool("rwt", 3)        # [128, 512] f32r rw1 k-tiles
    rsp = pool("rwts", 2)       # [128, 512] f32 staging for rw1
    bpp = pool("bpg", 2)        # [G, DS] per-group bias
    b1k = pool("big1k", 2)      # [128, B] scratch (bei / squares)
    ctp = pool("ctmp", 1)       # [128, 512] scratch
    stp = pool("stat", 10)      # [128, 1] BN stats smalls
    psp = pool("ps", 8, space="PSUM")
    drp = pool("dram", 1, space="DRAM")

    def ps_tile(p, n=BSL):
        return psp.tile([p, n], f32, tag="ps", name="ps")

    # ---- constants ----
    ones7 = cp.tile([G, BSL], f32r, tag="ones7", name="ones7")
    nc.gpsimd.dma_start(ones7[:], ones_d[0:G, :])
    ones1 = cp.tile([1, 128], f32r, tag="ones1", name="ones1")
    nc.gpsimd.dma_start(ones1[:], ones_d[0:1, 0:128])
    onesb = cp.tile([128, 1], f32r, tag="onesb", name="onesb")
    nc.gpsimd.dma_start(onesb[:], ones_d[:, 0:1])
    twosb = cp.tile([128, 1], f32r, tag="twosb", name="twosb")
    nc.gpsimd.dma_start(twosb[:], twos_d[:])
    sel = cp.tile([11, 11, 128], f32r, tag="sel", name="sel")
    nc.gpsimd.dma_start(sel[:], sel_d[:])
    bib = cp.tile([G, DS], f32r, tag="bib", name="bib")
    nc.gpsimd.dma_start(bib[:], bib_d[:])
    rb3 = cp.tile([11, 1], f32, tag="rb3", name="rb3")
    nc.sync.dma_start(rb3[:], rb3_d[:])
    rw2 = cp.tile([128, 4, 100], f32r, tag="rw2", name="rw2")
    nc.gpsimd.dma_start(rw2[:], rw2_d[:])
    rw3 = cp.tile([100, 11], f32r, tag="rw3", name="rw3")
    nc.gpsimd.dma_start(rw3[:], rw3_d[:])
    xu = []
    for ci, (m0, msz) in enumerate(CHUNKS):
        t = cp.tile([msz, B], f32, tag=f"xu{ci}", name=f"xu{ci}")
        nc.sync.dma_start(t[:], xuT_d[m0:m0 + msz, :])
        xu.append(t)

    # ---- persistent buffers ----
    h1 = [cp.tile([128, B], f32r, tag=f"h1_{m}", name=f"h1_{m}") for m in range(4)]
    gsb = [cp.tile([msz, 3 * B], f32, tag=f"gsb{ci}", name=f"gsb{ci}")
           for ci, (m0, msz) in enumerate(CHUNKS)]
    A = [cp.tile([msz, B], f32, tag=f"A{ci}", name=f"A{ci}") for ci, (m0, msz) in enumerate(CHUNKS)]
    z = [cp.tile([msz, B], f32, tag=f"z{ci}", name=f"z{ci}") for ci, (m0, msz) in enumerate(CHUNKS)]
    outsb = [cp.tile([msz, B], f32, tag=f"out{ci}", name=f"out{ci}") for ci, (m0, msz) in enumerate(CHUNKS)]
    h2sb = cp.tile([100, B], f32r, tag="h2sb", name="h2sb")
    ex10 = cp.tile([11, B], f32r, tag="ex10", name="ex10")
    arv = cp.tile([1, 3 * B], f32, tag="arv", name="arv")
    e10v = cp.tile([1, B], f32, tag="e10v", name="e10v")
    tv = cp.tile([1, B], f32r, tag="tv", name="tv")
    uv = cp.tile([1, B], f32r, tag="uv", name="uv")

    cc1_in = drp.tile([1, B], f32, tag="cc1_in", name="cc1_in")
    cc1_out = drp.tile([1, B], f32, tag="cc1_out", addr_space="Shared", name="cc1_out")
    cc2_in = drp.tile([1, 2 * B], f32, tag="cc2_in", name="cc2_in")
    cc2_out = drp.tile([1, 2 * B], f32, tag="cc2_out", addr_space="Shared", name="cc2_out")

    def stream_group(xsrc, wsrc):
        """DMA one group's x k-tiles + w k-tiles, reduce w over e.

        Both x and w stream f32 over HWDGE; x is rounded to f32r by an
        ACT copy (SWDGE dtype-cast DMA measured ~2x slower per byte).
        """
        xts, wss = [], []
        for kt in range(KT):
            ks = slice(kt * 128, (kt + 1) * 128)
            xs = xsp.tile([128, B], f32, tag="xts", name="xts")
            nc.sync.dma_start(xs[:], xsrc[ks, :])
            xt = xtp.tile([128, B], f32r, tag="xt", name="xt")
            nc.scalar.copy(xt[:], xs[:])
            xts.append(xt)
            wt = wtp.tile([128, DS, G], f32, tag="wt", name="wt")
            nc.sync.dma_start(wt[:], wsrc[ks, :, :])
            ws = wsp.tile([128, DS], f32r, tag="ws", name="ws")
            nc.vector.reduce_sum(out=ws[:], in_=wt[:], axis=AX.X)
            wss.append(ws)
        return xts, wss

    def group_matmuls(xts, wss, bias_lhs, copy_to=None, fold_ei=None):
        """Accumulate raw = x @ Wsum + sum_e bias into 4 psum tiles.

        copy_to: list of per-chunk (tile, free_offset) to store raw via ACT.
        fold_ei: ex10 row index -> fold e_i * raw into A instead of storing.
        """
        pss = {}
        for ci, (m0, msz) in enumerate(CHUNKS):
            for bs in range(NBS):
                pss[ci, bs] = ps_tile(msz)
        for kt in range(KT):
            for ci, (m0, msz) in enumerate(CHUNKS):
                for bs in range(NBS):
                    sl = slice(bs * BSL, (bs + 1) * BSL)
                    nc.tensor.matmul(
                        pss[ci, bs][:],
                        lhsT=wss[kt][:, m0:m0 + msz],
                        rhs=xts[kt][:, sl],
                        start=(kt == 0), stop=False)
        for ci, (m0, msz) in enumerate(CHUNKS):
            for bs in range(NBS):
                nc.tensor.matmul(
                    pss[ci, bs][:],
                    lhsT=bias_lhs[:, m0:m0 + msz],
                    rhs=ones7[:, 0:BSL],
                    start=False, stop=True)
        if copy_to is not None:
            for ci, (m0, msz) in enumerate(CHUNKS):
                dst, off = copy_to[ci]
                for bs in range(NBS):
                    nc.scalar.copy(dst[:, off + bs * BSL:off + (bs + 1) * BSL],
                                   pss[ci, bs][:])
        if fold_ei is not None:
            i, first = fold_ei
            for bs in range(NBS):
                sl = slice(bs * BSL, (bs + 1) * BSL)
                bc = ps_tile(128)
                nc.tensor.matmul(bc[:], lhsT=sel[:, i, :],
                                 rhs=ex10[:, sl], start=True, stop=True)
                bei = b1k.tile([128, B], f32, tag="big1k", name="big1k")
                nc.scalar.copy(bei[:, 0:BSL], bc[:])
                for ci, (m0, msz) in enumerate(CHUNKS):
                    tmp = ctp.tile([128, BSL], f32, tag="ctmp", name="ctmp")
                    nc.vector.tensor_tensor(out=tmp[0:msz, :], in0=pss[ci, bs][:],
                                            in1=bei[0:msz, 0:BSL], op=ALU.mult)
                    if first:
                        nc.vector.tensor_copy(A[ci][:, sl], tmp[0:msz, :])
                    else:
                        nc.vector.tensor_tensor(out=A[ci][:, sl], in0=A[ci][:, sl],
                                                in1=tmp[0:msz, :], op=ALU.add)
        return pss

    def fold_group(pss, i):
        """Fold e_i * raw (held in pss) into A."""
        for bs in range(NBS):
            sl = slice(bs * BSL, (bs + 1) * BSL)
            bc = ps_tile(128)
            nc.tensor.matmul(bc[:], lhsT=sel[:, i, :],
                             rhs=ex10[:, sl], start=True, stop=True)
            bei = b1k.tile([128, B], f32, tag="big1k", name="big1k")
            nc.scalar.copy(bei[:, 0:BSL], bc[:])
            for ci, (m0, msz) in enumerate(CHUNKS):
                tmp = ctp.tile([128, BSL], f32, tag="ctmp", name="ctmp")
                nc.vector.tensor_tensor(out=tmp[0:msz, :], in0=pss[ci, bs][:],
                                        in1=bei[0:msz, 0:BSL], op=ALU.mult)
                nc.vector.tensor_tensor(out=A[ci][:, sl], in0=A[ci][:, sl],
                                        in1=tmp[0:msz, :], op=ALU.add)

    # A starts at zero; all folds (late + stored) accumulate into it.
    for ci in range(len(CHUNKS)):
        nc.vector.memset(A[ci][:], 0.0)

    # ================= uni branch first -> z, ssz partial, AR1 =========
    xibts, wibss = stream_group(xibT_d, wib_d)
    upss = group_matmuls(xibts, wibss, bib)
    for ci, (m0, msz) in enumerate(CHUNKS):
        for bs in range(NBS):
            sl = slice(bs * BSL, (bs + 1) * BSL)
            nc.scalar.mul(z[ci][:, sl], upss[ci, bs][:], 0.1 / 7.0)
            t9 = ctp.tile([128, BSL], f32, tag="ctmp", name="ctmp")
            nc.vector.tensor_scalar_mul(t9[0:msz, :], xu[ci][:, sl], 0.9)
            nc.vector.tensor_tensor(out=z[ci][:, sl], in0=z[ci][:, sl],
                                    in1=t9[0:msz, :], op=ALU.add)
    # ssz partial: sum_d z^2 -> arv[0, 2B:3B] -> cc1_in
    for bs in range(NBS):
        sl = slice(bs * BSL, (bs + 1) * BSL)
        ps = ps_tile(1)
        for ci, (m0, msz) in enumerate(CHUNKS):
            sq = b1k.tile([128, B], f32r, tag="big1k", name="big1k")
            nc.scalar.square(sq[0:msz, 0:BSL], z[ci][:, sl])
            nc.tensor.matmul(ps[:], lhsT=onesb[0:msz, :],
                             rhs=sq[0:msz, 0:BSL],
                             start=(ci == 0), stop=(ci == len(CHUNKS) - 1))
        nc.scalar.copy(arv[:, 2 * B + bs * BSL:2 * B + (bs + 1) * BSL], ps[:])
    nc.sync.dma_start(cc1_in[:], arv[:, 2 * B:3 * B])

    # ================= router groups (also feed h1) =================
    for ri, g in enumerate(ROUTER_GS):
        xts, wss = stream_group(xT_d[g], w_d[g])
        bp_g = bpp.tile([G, DS], f32r, tag="bpg", name="bpg")
        nc.gpsimd.dma_start(bp_g[:], bp_d[:, g, :])
        group_matmuls(xts, wss, bp_g,
                      copy_to=[(gsb[ci], ri * B) for ci in range(len(CHUNKS))])
        if ri == 0:
            # AR1 doorbell: emitted after g0's SWDGE casts so its input
            # (done well before) never blocks the gpsimd queue head.
            nc.gpsimd.collective_compute(
                "AllReduce", ALU.add,
                ins=[cc1_in.opt()], outs=[cc1_out.opt()],
                replica_groups=[list(range(NCORES))])
        # router h1 partial: h1T[m] (+)= rw1[g-block].T @ xT[g]
        rts = []
        for kt in range(KT):
            rs_ = rsp.tile([128, BSL], f32, tag="rwts", name="rwts")
            nc.sync.dma_start(rs_[:], rw1_d[(ri * KT + kt) * 128:(ri * KT + kt + 1) * 128, :])
            rt = rwp.tile([128, BSL], f32r, tag="rwt", name="rwt")
            nc.scalar.copy(rt[:], rs_[:])
            rts.append(rt)
        for m in range(4):
            for bs in range(NBS):
                sl = slice(bs * BSL, (bs + 1) * BSL)
                ps = ps_tile(128)
                for kt in range(KT):
                    nc.tensor.matmul(
                        ps[:],
                        lhsT=rts[kt][:, m * 128:(m + 1) * 128],
                        rhs=xts[kt][:, sl],
                        start=(kt == 0), stop=(kt == KT - 1))
                if ri == 0:
                    nc.scalar.copy(h1[m][:, sl], ps[:])
                else:
                    nc.vector.tensor_tensor(out=h1[m][:, sl], in0=ps[:],
                                            in1=h1[m][:, sl], op=ALU.add)

    # ======= stream group 1 before the finalize (overlap its DMA/PE) ===
    xts1, wss1 = stream_group(xT_d[1], w_d[1])
    bp_1 = bpp.tile([G, DS], f32r, tag="bpg", name="bpg")
    nc.gpsimd.dma_start(bp_1[:], bp_d[:, 1, :])
    pss1 = group_matmuls(xts1, wss1, bp_1)
    # group 2's stream (no PSUM use) also overlaps the finalize window
    xts2, wss2 = stream_group(xT_d[2], w_d[2])
    bp_2 = bpp.tile([G, DS], f32r, tag="bpg", name="bpg")
    nc.gpsimd.dma_start(bp_2[:], bp_d[:, 2, :])

    # ================= router finalize =================
    def bn_act(tiles, nparts, func):
        """BatchNorm (training stats over free axis) + activation, in-place."""
        for t in tiles:
            dump = b1k.tile([128, B], f32, tag="big1k", name="big1k")
            mnr = stp.tile([128, 1], f32, tag="stat", name="stat")
            nc.scalar.activation(dump[0:nparts, :], t[:], ACTF.Copy,
                                 accum_out=mnr[0:nparts, :])
            mn = stp.tile([128, 1], f32, tag="stat", name="stat")
            nc.scalar.mul(mn[0:nparts, :], mnr[0:nparts, :], 1.0 / B)
            sq = b1k.tile([128, B], f32, tag="big1k", name="big1k")
            ex2r = stp.tile([128, 1], f32, tag="stat", name="stat")
            nc.scalar.activation(sq[0:nparts, :], t[:], ACTF.Square,
                                 accum_out=ex2r[0:nparts, :])
            ex2 = stp.tile([128, 1], f32, tag="stat", name="stat")
            nc.scalar.mul(ex2[0:nparts, :], ex2r[0:nparts, :], 1.0 / B)
            var = stp.tile([128, 1], f32, tag="stat", name="stat")
            nc.vector.tensor_tensor(out=var[0:nparts, :], in0=mn[0:nparts, :],
                                    in1=mn[0:nparts, :], op=ALU.mult)
            nc.vector.tensor_tensor(out=var[0:nparts, :], in0=ex2[0:nparts, :],
                                    in1=var[0:nparts, :], op=ALU.subtract)
            nc.vector.tensor_scalar_add(var[0:nparts, :], var[0:nparts, :], EPS_BN)
            sd = stp.tile([128, 1], f32, tag="stat", name="stat")
            nc.scalar.sqrt(sd[0:nparts, :], var[0:nparts, :])
            rs = stp.tile([128, 1], f32, tag="stat", name="stat")
            nc.vector.reciprocal(rs[0:nparts, :], sd[0:nparts, :])
            nb = stp.tile([128, 1], f32, tag="stat", name="stat")
            nc.vector.tensor_tensor(out=nb[0:nparts, :], in0=mn[0:nparts, :],
                                    in1=rs[0:nparts, :], op=ALU.mult)
            nc.vector.tensor_scalar_mul(nb[0:nparts, :], nb[0:nparts, :], -1.0)
            nc.scalar.activation(t[:], t[:], func,
                                 bias=nb[0:nparts, :], scale=rs[0:nparts, :])

    bn_act(h1, 128, ACTF.Relu)
    for bs in range(NBS):
        sl = slice(bs * BSL, (bs + 1) * BSL)
        ps = ps_tile(100)
        for kt in range(4):
            nc.tensor.matmul(ps[:], lhsT=rw2[:, kt, :],
                             rhs=h1[kt][:, sl],
                             start=(kt == 0), stop=(kt == 3))
        nc.scalar.copy(h2sb[:, sl], ps[:])
    bn_act([h2sb], 100, ACTF.Tanh)
    for bs in range(NBS):
        sl = slice(bs * BSL, (bs + 1) * BSL)
        ps = ps_tile(11)
        nc.tensor.matmul(ps[:], lhsT=rw3[:],
                         rhs=h2sb[:, sl], start=True, stop=True)
        sg = ctp.tile([128, BSL], f32, tag="ctmp", name="ctmp")
        nc.scalar.activation(sg[0:11, :], ps[:], ACTF.Sigmoid, bias=rb3[:], scale=1.0)
        nc.scalar.activation(ex10[:, sl], sg[0:11, :], ACTF.Exp, scale=10.0)

    # fetch AR1 result (ssz sum); emitted here so the sync-queue wait
    # lands long after the collective completed.
    nc.sync.dma_start(arv[:, 2 * B:3 * B], cc1_out[:])

    # ================= e10 row, s-chain, g1 + stored folds ============
    for bs in range(NBS):
        sl = slice(bs * BSL, (bs + 1) * BSL)
        bc = ps_tile(128)
        nc.tensor.matmul(bc[:], lhsT=sel[:, 10, :],
                         rhs=ex10[:, sl], start=True, stop=True)
        nc.scalar.copy(e10v[:, sl], bc[0:1, :])
    # e10v holds 7*e10 from here on
    nc.vector.tensor_scalar_mul(e10v[:], e10v[:], 7.0)
    s_v = arv[:, 2 * B:3 * B]
    nc.scalar.sqrt(s_v, s_v)
    nc.vector.tensor_scalar_max(s_v, s_v, EPS_NORM)
    nc.vector.reciprocal(uv[:], s_v)
    nc.vector.tensor_tensor(out=tv[:], in0=e10v[:], in1=uv[:], op=ALU.mult)
    # e10v now holds (7*e10)^2 -- consumed in the post-AR2 norm only
    nc.vector.tensor_tensor(out=e10v[:], in0=e10v[:], in1=e10v[:], op=ALU.mult)

    fold_group(pss1, 1)
    for ri2, g2 in enumerate(ROUTER_GS):
        for bs in range(NBS):
            sl = slice(bs * BSL, (bs + 1) * BSL)
            bc = ps_tile(128)
            nc.tensor.matmul(bc[:], lhsT=sel[:, g2, :],
                             rhs=ex10[:, sl], start=True, stop=True)
            for ci, (m0, msz) in enumerate(CHUNKS):
                gsl = gsb[ci][:, ri2 * B + bs * BSL:ri2 * B + (bs + 1) * BSL]
                tmp = ctp.tile([128, BSL], f32, tag="ctmp", name="ctmp")
                nc.vector.tensor_tensor(out=tmp[0:msz, :], in0=gsl,
                                        in1=bc[0:msz, :], op=ALU.mult)
                nc.vector.tensor_tensor(out=A[ci][:, sl], in0=A[ci][:, sl],
                                        in1=tmp[0:msz, :], op=ALU.add)

    # group 2: matmuls + fold (stream already issued above)
    pss2 = group_matmuls(xts2, wss2, bp_2)
    fold_group(pss2, 2)

    # ================= remaining late groups: stream + fold ===========
    for g in LATE_GS[2:]:
        xts, wss = stream_group(xT_d[g], w_d[g])
        bp_g = bpp.tile([G, DS], f32r, tag="bpg", name="bpg")
        nc.gpsimd.dma_start(bp_g[:], bp_d[:, g, :])
        pss_g = group_matmuls(xts, wss, bp_g)
        fold_group(pss_g, g)

    # ================= norm partials a, c -> AR2 =================
    for bs in range(NBS):
        sl = slice(bs * BSL, (bs + 1) * BSL)
        psa = ps_tile(1)
        psc = ps_tile(1)
        for ci, (m0, msz) in enumerate(CHUNKS):
            sqa = b1k.tile([128, B], f32r, tag="big1k", name="big1k")
            nc.scalar.square(sqa[0:msz, 0:BSL], A[ci][:, sl])
            nc.tensor.matmul(psa[:], lhsT=onesb[0:msz, :],
                             rhs=sqa[0:msz, 0:BSL],
                             start=(ci == 0), stop=(ci == len(CHUNKS) - 1))
            cza = b1k.tile([128, B], f32r, tag="big1k", name="big1k")
            nc.vector.tensor_tensor(out=cza[0:msz, 0:BSL], in0=A[ci][:, sl],
                                    in1=z[ci][:, sl], op=ALU.mult)
            nc.tensor.matmul(psc[:], lhsT=twosb[0:msz, :],
                             rhs=cza[0:msz, 0:BSL],
                             start=(ci == 0), stop=(ci == len(CHUNKS) - 1))
        nc.scalar.copy(arv[:, bs * BSL:(bs + 1) * BSL], psa[:])
        nc.scalar.copy(arv[:, B + bs * BSL:B + (bs + 1) * BSL], psc[:])

    nc.sync.dma_start(cc2_in[:], arv[:, 0:2 * B])
    nc.gpsimd.collective_compute(
        "AllReduce", ALU.add,
        ins=[cc2_in.opt()], outs=[cc2_out.opt()],
        replica_groups=[list(range(NCORES))])

    # P = A + t*z, computed while AR2 is in flight (tv is known early).
    # A's old value was already consumed by the a/c partials above.
    for bs in range(NBS):
        sl = slice(bs * BSL, (bs + 1) * BSL)
        btv = ps_tile(128)
        nc.tensor.matmul(btv[:], lhsT=ones1[:],
                         rhs=tv[:, sl], start=True, stop=True)
        for ci, (m0, msz) in enumerate(CHUNKS):
            t2 = ctp.tile([128, BSL], f32, tag="ctmp", name="ctmp")
            nc.vector.tensor_tensor(out=t2[0:msz, :], in0=z[ci][:, sl],
                                    in1=btv[0:msz, :], op=ALU.mult)
            nc.vector.tensor_tensor(out=A[ci][:, sl], in0=A[ci][:, sl],
                                    in1=t2[0:msz, :], op=ALU.add)

    nc.sync.dma_start(arv[:, 0:2 * B], cc2_out[:])

    # ================= final scalars and output =================
    a_v = arv[:, 0:B]
    c_v = arv[:, B:2 * B]   # holds 2*sum_d(A*z) via the twos lhsT
    # q = a + t*(2c) + (7*e10)^2 ; u = 1/max(sqrt(q), eps); out = P*u
    nc.vector.tensor_tensor(out=c_v, in0=tv[:], in1=c_v, op=ALU.mult)
    nc.vector.tensor_tensor(out=a_v, in0=a_v, in1=c_v, op=ALU.add)
    nc.vector.tensor_tensor(out=a_v, in0=a_v, in1=e10v[:], op=ALU.add)
    nc.scalar.sqrt(a_v, a_v)
    nc.vector.tensor_scalar_max(a_v, a_v, EPS_NORM)
    nc.vector.reciprocal(uv[:], a_v)
    for bs in range(NBS):
        sl = slice(bs * BSL, (bs + 1) * BSL)
        bu = ps_tile(128)
        nc.tensor.matmul(bu[:], lhsT=ones1[:],
                         rhs=uv[:, sl], start=True, stop=True)
        for ci, (m0, msz) in enumerate(CHUNKS):
            nc.vector.tensor_tensor(out=outsb[ci][:, sl], in0=A[ci][:, sl],
                                    in1=bu[0:msz, :], op=ALU.mult)
    for ci, (m0, msz) in enumerate(CHUNKS):
        nc.sync.dma_start(outT_d[m0:m0 + msz, :], outsb[ci][:])

    for p in reversed(list(pools.values())):
        p.__exit__(None, None, None)


def _build_nc():
    nc = bacc.Bacc("TRN2", target_bir_lowering=False, debug=False,
                   num_devices=NCORES)
    with tile.TileContext(nc) as tc:
        with nc.allow_low_precision(reason="fp32r PE inputs are intentionally rounded"):
            _emit(nc, tc)
    nc.compile()
    return nc


def _host_prep(inputs):
    x_enc = np.ascontiguousarray(np.asarray(inputs["x_enc"], dtype=np.float32))
    x_ib = np.asarray(inputs["x_ib"], dtype=np.float32)
    x_uni = np.asarray(inputs["x_uni"], dtype=np.float32)
    W_proj = np.asarray(inputs["W_proj"], dtype=np.float32)
    b_proj = np.asarray(inputs["b_proj"], dtype=np.float32)
    W_ib = np.asarray(inputs["W_ib"], dtype=np.float32)
    b_ib = np.asarray(inputs["b_ib"], dtype=np.float32)

    xT = np.ascontiguousarray(x_enc.transpose(0, 2, 1))          # [N, K, B]
    xibT = np.ascontiguousarray(x_ib.T)                          # [K, B]
    sel = np.zeros((11, 11, 128), dtype=np.float32)
    for q in range(11):
        sel[q, q, :] = 1.0
    rb3 = np.ascontiguousarray(np.asarray(inputs["r_b3"], np.float32).reshape(11, 1))
    rw1 = np.ascontiguousarray(np.asarray(inputs["r_w1"], np.float32))
    rw2 = np.ascontiguousarray(
        np.asarray(inputs["r_w2"], np.float32).reshape(4, 128, 100).transpose(1, 0, 2))
    rw3 = np.ascontiguousarray(np.asarray(inputs["r_w3"], np.float32))
    ones_host = np.ones((128, BSL), dtype=np.float32)
    twos_host = np.full((128, 1), 2.0, dtype=np.float32)

    in_maps = []
    for c in range(NCORES):
        ds = slice(c * DS, (c + 1) * DS)
        in_maps.append({
            "xT": xT,
            "xibT": xibT,
            "w": np.ascontiguousarray(W_proj[:, :, :, ds].transpose(0, 2, 3, 1)),
            "wib": np.ascontiguousarray(W_ib[:, :, ds].transpose(1, 2, 0)),
            "xuT": np.ascontiguousarray(x_uni[:, ds].T),
            "bp": np.ascontiguousarray(b_proj[:, :, ds].transpose(1, 0, 2)),
            "bib": np.ascontiguousarray(b_ib[:, ds]),
            "rw1": rw1,
            "rw2": rw2,
            "rw3": rw3,
            "rb3": rb3,
            "sel": sel,
            "onesd": ones_host,
            "twosd": twos_host,
        })
    return in_maps


def kernel(**inputs):
    global LAST_RESULTS
    if "nc" not in _NC_CACHE:
        _NC_CACHE["nc"] = _build_nc()
    nc = _NC_CACHE["nc"]
    in_maps = _host_prep(inputs)
    res = run_bass_kernel_spmd(nc, in_maps, list(range(NCORES)))
    LAST_RESULTS = res
    full = np.concatenate([res.results[c]["outT"] for c in range(NCORES)], axis=0)
    return np.ascontiguousarray(full.T)



# revision 4
# speedup vs baseline: 1.1188x; 1.1188x over previous
"""Trainium2 Bass kernel for nn_ATVP_router_UNI (moe_routing).

Sharding: output dim D=1536 sharded over 8 cores (192 each). Activations
(x_enc, x_ib) are broadcast in a partition-major transposed layout; the
router MLP is replicated on every core. The expert mean over e commutes
with the linear projection, so each core streams its W_proj slice once
(bf16), reduces over e on DVE, and runs 1/7 of the naive matmul FLOPs in
bf16. All heavy streams (W_proj, x_enc, W_ib, x_ib, r_w1, x_uni) are cast
to bf16 on the host -- this halves HBM traffic, which is the roofline for
this kernel. The softmax denominator and the 1/7 group-mean scale cancel
inside the final l2 normalization and are folded away. Cross-core coupling
is two 4KB AllReduces: the uni-branch sum-of-squares (fired early, hidden
under streaming) and the final out-norm partials on the tail.

Per-group raw projections are staged to SBUF immediately (bf16), which
decouples PSUM from the router-dependent gated folds and keeps the weight
stream saturated end-to-end.

kernel(**inputs) takes the full unsharded inputs and returns the full
[1024, 1536] f32 output. Host-side prep does layout/dtype staging only --
all arithmetic runs on device.
"""

import numpy as np
import ml_dtypes

import concourse.bass as bass
import concourse.tile as tile
import concourse.mybir as mybir
from concourse import bacc
from concourse.bass_utils import run_bass_kernel_spmd

f32 = mybir.dt.float32
f32r = mybir.dt.float32r
bf16 = mybir.dt.bfloat16
AX = mybir.AxisListType
ALU = mybir.AluOpType
ACTF = mybir.ActivationFunctionType

NCORES = 8
B, N, G, K, D = 1024, 10, 7, 1024, 1536
DS = D // NCORES          # 192 output dims per core
KT = K // 128             # 8 k-tiles
KH = KT // 2              # 4 k-tiles per W half-transfer
BSL = 512                 # free-dim slice for matmuls
NBS = B // BSL            # 2
ROUTER_GS = (0, 4, 8)     # TEXT_PRED groups, streamed first
STREAM_GS = (0, 4, 8, 1, 2, 3, 5, 6, 7, 9)
CHUNKS = ((0, 128), (128, 64))  # (d-offset, size) chunks of DS=192
EPS_BN = 1e-5
EPS_NORM = 1e-12

LAST_RESULTS = None
_NC_CACHE = {}


def _emit(nc, tc):
    # ---- DRAM I/O ----
    xT_d = nc.dram_tensor("xT", [N, 128, KT, B], bf16, kind="ExternalInput").ap()
    xibT_d = nc.dram_tensor("xibT", [128, KT, B], bf16, kind="ExternalInput").ap()
    w_d = nc.dram_tensor("w", [N, 128, KT, DS, G], bf16, kind="ExternalInput").ap()
    wib_d = nc.dram_tensor("wib", [128, KT, DS, G], bf16, kind="ExternalInput").ap()
    xuT_d = nc.dram_tensor("xuT", [DS, B], bf16, kind="ExternalInput").ap()
    bp_d = nc.dram_tensor("bp", [G, N, DS], bf16, kind="ExternalInput").ap()
    bib_d = nc.dram_tensor("bib", [G, DS], bf16, kind="ExternalInput").ap()
    rw1_d = nc.dram_tensor("rw1", [3, 128, KT, BSL], bf16, kind="ExternalInput").ap()
    rw2_d = nc.dram_tensor("rw2", [128, 4, 100], bf16, kind="ExternalInput").ap()
    rw3_d = nc.dram_tensor("rw3", [100, 11], bf16, kind="ExternalInput").ap()
    rb3_d = nc.dram_tensor("rb3", [11, 1], f32, kind="ExternalInput").ap()
    sel_d = nc.dram_tensor("sel", [11, 11, 128], f32, kind="ExternalInput").ap()
    ones_d = nc.dram_tensor("onesd", [128, BSL], f32, kind="ExternalInput").ap()
    outT_d = nc.dram_tensor("outT", [DS, B], f32, kind="ExternalOutput").ap()

    pools = {}

    def pool(name, bufs, space="SBUF"):
        cm = tc.tile_pool(name=name, bufs=bufs, space=space)
        pools[name] = cm
        return cm.__enter__()

    cp = pool("const", 1)       # persistent constants / accumulators
    xtp = pool("xt", 2)         # [128, KT, B] bf16 whole-group x
    wtp = pool("wt", 2)         # [128, KH, DS, G] bf16 half-group W
    wsp = pool("ws", 2)         # [128, KT, DS] bf16 e-reduced weights
    gs0 = pool("gs0", 4)        # [128, B] bf16 staged raw (chunk 0)
    gs1 = pool("gs1", 4)        # [64, B] bf16 staged raw (chunk 1)
    rwp = pool("rwt", 2)        # [128, KT, BSL] bf16 rw1 per router group
    bpp = pool("bpg", 2)        # [G, DS] bf16 per-group bias
    b1k = pool("big1k", 2)      # [128, B] scratch
    ctp = pool("ctmp", 1)       # [128, BSL] scratch
    stp = pool("stat", 10)      # [128, 1] BN stats smalls
    psp = pool("ps", 8, space="PSUM")
    drp = pool("dram", 1, space="DRAM")

    def ps_tile(p, n=BSL):
        return psp.tile([p, n], f32, tag="ps", name="ps")

    # ---- constants ----
    ones7 = cp.tile([G, BSL], bf16, tag="ones7", name="ones7")
    nc.gpsimd.dma_start(ones7[:], ones_d[0:G, :])
    ones1 = cp.tile([1, 128], f32r, tag="ones1", name="ones1")
    nc.gpsimd.dma_start(ones1[:], ones_d[0:1, 0:128])
    onesb = cp.tile([128, 1], f32r, tag="onesb", name="onesb")
    nc.gpsimd.dma_start(onesb[:], ones_d[:, 0:1])
    sel = cp.tile([11, 11, 128], f32r, tag="sel", name="sel")
    nc.gpsimd.dma_start(sel[:], sel_d[:])
    bib = cp.tile([G, DS], bf16, tag="bib", name="bib")
    nc.gpsimd.dma_start(bib[:], bib_d[:])
    rb3 = cp.tile([11, 1], f32, tag="rb3", name="rb3")
    nc.sync.dma_start(rb3[:], rb3_d[:])
    rw2 = cp.tile([128, 4, 100], bf16, tag="rw2", name="rw2")
    nc.gpsimd.dma_start(rw2[:], rw2_d[:])
    rw3 = cp.tile([100, 11], bf16, tag="rw3", name="rw3")
    nc.gpsimd.dma_start(rw3[:], rw3_d[:])
    xu = []
    for ci, (m0, msz) in enumerate(CHUNKS):
        t = cp.tile([msz, B], bf16, tag=f"xu{ci}", name=f"xu{ci}")
        nc.sync.dma_start(t[:], xuT_d[m0:m0 + msz, :])
        xu.append(t)

    # ---- persistent buffers ----
    h1 = [cp.tile([128, B], f32, tag=f"h1_{m}", name=f"h1_{m}") for m in range(4)]
    h1b = [cp.tile([128, B], bf16, tag=f"h1b_{m}", name=f"h1b_{m}") for m in range(4)]
    A = [cp.tile([msz, B], f32, tag=f"A{ci}", name=f"A{ci}") for ci, (m0, msz) in enumerate(CHUNKS)]
    z = [cp.tile([msz, B], f32, tag=f"z{ci}", name=f"z{ci}") for ci, (m0, msz) in enumerate(CHUNKS)]
    outsb = [cp.tile([msz, B], f32, tag=f"out{ci}", name=f"out{ci}") for ci, (m0, msz) in enumerate(CHUNKS)]
    h2sb = cp.tile([100, B], f32, tag="h2sb", name="h2sb")
    h2b = cp.tile([100, B], bf16, tag="h2b", name="h2b")
    ex10 = cp.tile([11, B], f32r, tag="ex10", name="ex10")
    arv = cp.tile([1, 2 * B], f32, tag="arv", name="arv")
    tv = cp.tile([1, B], f32r, tag="tv", name="tv")
    uv = cp.tile([1, B], f32r, tag="uv", name="uv")

    cc1_in = drp.tile([1, B], f32, tag="cc1_in", name="cc1_in")
    cc1_out = drp.tile([1, B], f32, tag="cc1_out", addr_space="Shared", name="cc1_out")
    cc2_in = drp.tile([1, B], f32, tag="cc2_in", name="cc2_in")
    cc2_out = drp.tile([1, B], f32, tag="cc2_out", addr_space="Shared", name="cc2_out")

    for ci in range(len(CHUNKS)):
        nc.vector.memset(A[ci][:], 0.0)

    def stream_group(xsrc, wsrc):
        """DMA one group's x (one shot) + W (two halves), reduce W over e."""
        xt = xtp.tile([128, KT, B], bf16, tag="xt", name="xt")
        nc.sync.dma_start(xt[:], xsrc)
        ws = wsp.tile([128, KT, DS], bf16, tag="ws", name="ws")
        for h in range(2):
            wt = wtp.tile([128, KH, DS, G], bf16, tag="wt", name="wt")
            nc.sync.dma_start(wt[:], wsrc[:, h * KH:(h + 1) * KH, :, :])
            nc.vector.reduce_sum(out=ws[:, h * KH:(h + 1) * KH, :], in_=wt[:],
                                 axis=AX.X)
        return xt, ws

    def group_matmuls(xt, ws, bias_lhs):
        """raw = x @ Wsum + sum_e bias; stage to SBUF bf16 tiles."""
        raws = []
        for ci, (m0, msz) in enumerate(CHUNKS):
            gp = gs0 if ci == 0 else gs1
            raw = gp.tile([msz, B], bf16, tag=f"gs{ci}", name=f"gs{ci}")
            for bs in range(NBS):
                sl = slice(bs * BSL, (bs + 1) * BSL)
                ps = ps_tile(msz)
                for kt in range(KT):
                    nc.tensor.matmul(
                        ps[:],
                        lhsT=ws[:, kt, m0:m0 + msz],
                        rhs=xt[:, kt, sl],
                        start=(kt == 0), stop=False)
                nc.tensor.matmul(
                    ps[:],
                    lhsT=bias_lhs[:, m0:m0 + msz],
                    rhs=ones7[:, 0:BSL],
                    start=False, stop=True)
                nc.scalar.copy(raw[:, sl], ps[:])
            raws.append(raw)
        return raws

    def fold_group(raws, i):
        """A += e_i * raw (e broadcast across partitions via sel matmul)."""
        for bs in range(NBS):
            sl = slice(bs * BSL, (bs + 1) * BSL)
            bc = ps_tile(128)
            nc.tensor.matmul(bc[:], lhsT=sel[:, i, :],
                             rhs=ex10[:, sl], start=True, stop=True)
            for ci, (m0, msz) in enumerate(CHUNKS):
                tmp = ctp.tile([128, BSL], f32, tag="ctmp", name="ctmp")
                nc.vector.tensor_tensor(out=tmp[0:msz, :], in0=raws[ci][:, sl],
                                        in1=bc[0:msz, :], op=ALU.mult)
                nc.vector.tensor_tensor(out=A[ci][:, sl], in0=A[ci][:, sl],
                                        in1=tmp[0:msz, :], op=ALU.add)

    # ================= uni branch first -> z, ssz partial, AR1 =========
    xib_t, wib_s = stream_group(xibT_d, wib_d)
    for ci, (m0, msz) in enumerate(CHUNKS):
        for bs in range(NBS):
            sl = slice(bs * BSL, (bs + 1) * BSL)
            ps = ps_tile(msz)
            for kt in range(KT):
                nc.tensor.matmul(ps[:], lhsT=wib_s[:, kt, m0:m0 + msz],
                                 rhs=xib_t[:, kt, sl],
                                 start=(kt == 0), stop=False)
            nc.tensor.matmul(ps[:], lhsT=bib[:, m0:m0 + msz],
                             rhs=ones7[:, 0:BSL], start=False, stop=True)
            nc.scalar.mul(z[ci][:, sl], ps[:], 0.1 / 7.0)
            t9 = ctp.tile([128, BSL], f32, tag="ctmp", name="ctmp")
            nc.vector.tensor_scalar_mul(t9[0:msz, :], xu[ci][:, sl], 0.9)
            nc.vector.tensor_tensor(out=z[ci][:, sl], in0=z[ci][:, sl],
                                    in1=t9[0:msz, :], op=ALU.add)
    # ssz partial: sum_d z^2 -> arv[0, B:2B] -> cc1_in
    for bs in range(NBS):
        sl = slice(bs * BSL, (bs + 1) * BSL)
        ps = ps_tile(1)
        for ci, (m0, msz) in enumerate(CHUNKS):
            sq = b1k.tile([128, B], f32r, tag="big1k", name="big1k")
            nc.scalar.square(sq[0:msz, 0:BSL], z[ci][:, sl])
            nc.tensor.matmul(ps[:], lhsT=onesb[0:msz, :],
                             rhs=sq[0:msz, 0:BSL],
                             start=(ci == 0), stop=(ci == len(CHUNKS) - 1))
        nc.scalar.copy(arv[:, B + bs * BSL:B + (bs + 1) * BSL], ps[:])
    nc.sync.dma_start(cc1_in[:], arv[:, B:2 * B])

    # ================= router groups (also feed h1) =================
    router_raws = {}
    for ri, g in enumerate(ROUTER_GS):
        xt, ws = stream_group(xT_d[g], w_d[g])
        bp_g = bpp.tile([G, DS], bf16, tag="bpg", name="bpg")
        nc.gpsimd.dma_start(bp_g[:], bp_d[:, g, :])
        router_raws[g] = group_matmuls(xt, ws, bp_g)
        if ri == 0:
            # AR1 doorbell: emitted here so its gpsimd-queue wait (on the
            # early cc1_in DMA) never blocks later gpsimd work.
            nc.gpsimd.collective_compute(
                "AllReduce", ALU.add,
                ins=[cc1_in.opt()], outs=[cc1_out.opt()],
                replica_groups=[list(range(NCORES))])
        # router h1 partial: h1[m] (+)= rw1[g-block].T @ xT[g]
        rt = rwp.tile([128, KT, BSL], bf16, tag="rwt", name="rwt")
        nc.sync.dma_start(rt[:], rw1_d[ri])
        for m in range(4):
            for bs in range(NBS):
                sl = slice(bs * BSL, (bs + 1) * BSL)
                ps = ps_tile(128)
                for kt in range(KT):
                    nc.tensor.matmul(
                        ps[:],
                        lhsT=rt[:, kt, m * 128:(m + 1) * 128],
                        rhs=xt[:, kt, sl],
                        start=(kt == 0), stop=(kt == KT - 1))
                if ri == 0:
                    nc.scalar.copy(h1[m][:, sl], ps[:])
                else:
                    nc.vector.tensor_tensor(out=h1[m][:, sl], in0=ps[:],
                                            in1=h1[m][:, sl], op=ALU.add)

    # ======= stream group 1 before the finalize (overlap its DMA/PE) ===
    g1 = STREAM_GS[3]
    xt1, ws1 = stream_group(xT_d[g1], w_d[g1])
    bp_1 = bpp.tile([G, DS], bf16, tag="bpg", name="bpg")
    nc.gpsimd.dma_start(bp_1[:], bp_d[:, g1, :])
    raws1 = group_matmuls(xt1, ws1, bp_1)

    # ================= router finalize =================
    def bn_act(tiles, out_tiles, nparts, func):
        """BatchNorm (training stats over free axis) + activation."""
        for t, to in zip(tiles, out_tiles):
            dump = b1k.tile([128, B], f32, tag="big1k", name="big1k")
            mnr = stp.tile([128, 1], f32, tag="stat", name="stat")
            nc.scalar.activation(dump[0:nparts, :], t[:], ACTF.Copy,
                                 accum_out=mnr[0:nparts, :])
            mn = stp.tile([128, 1], f32, tag="stat", name="stat")
            nc.scalar.mul(mn[0:nparts, :], mnr[0:nparts, :], 1.0 / B)
            sq = b1k.tile([128, B], f32, tag="big1k", name="big1k")
            ex2r = stp.tile([128, 1], f32, tag="stat", name="stat")
            nc.scalar.activation(sq[0:nparts, :], t[:], ACTF.Square,
                                 accum_out=ex2r[0:nparts, :])
            ex2 = stp.tile([128, 1], f32, tag="stat", name="stat")
            nc.scalar.mul(ex2[0:nparts, :], ex2r[0:nparts, :], 1.0 / B)
            var = stp.tile([128, 1], f32, tag="stat", name="stat")
            nc.vector.tensor_tensor(out=var[0:nparts, :], in0=mn[0:nparts, :],
                                    in1=mn[0:nparts, :], op=ALU.mult)
            nc.vector.tensor_tensor(out=var[0:nparts, :], in0=ex2[0:nparts, :],
                                    in1=var[0:nparts, :], op=ALU.subtract)
            nc.vector.tensor_scalar_add(var[0:nparts, :], var[0:nparts, :], EPS_BN)
            sd = stp.tile([128, 1], f32, tag="stat", name="stat")
            nc.scalar.sqrt(sd[0:nparts, :], var[0:nparts, :])
            rs = stp.tile([128, 1], f32, tag="stat", name="stat")
            nc.vector.reciprocal(rs[0:nparts, :], sd[0:nparts, :])
            nb = stp.tile([128, 1], f32, tag="stat", name="stat")
            nc.vector.tensor_tensor(out=nb[0:nparts, :], in0=mn[0:nparts, :],
                                    in1=rs[0:nparts, :], op=ALU.mult)
            nc.vector.tensor_scalar_mul(nb[0:nparts, :], nb[0:nparts, :], -1.0)
            nc.scalar.activation(to[:], t[:], func,
                                 bias=nb[0:nparts, :], scale=rs[0:nparts, :])

    bn_act(h1, h1b, 128, ACTF.Relu)
    for bs in range(NBS):
        sl = slice(bs * BSL, (bs + 1) * BSL)
        ps = ps_tile(100)
        for kt in range(4):
            nc.tensor.matmul(ps[:], lhsT=rw2[:, kt, :],
                             rhs=h1b[kt][:, sl],
                             start=(kt == 0), stop=(kt == 3))
        nc.scalar.copy(h2sb[:, sl], ps[:])
    bn_act([h2sb], [h2b], 100, ACTF.Tanh)
    for bs in range(NBS):
        sl = slice(bs * BSL, (bs + 1) * BSL)
        ps = ps_tile(11)
        nc.tensor.matmul(ps[:], lhsT=rw3[:],
                         rhs=h2b[:, sl], start=True, stop=True)
        sg = ctp.tile([128, BSL], f32, tag="ctmp", name="ctmp")
        nc.scalar.activation(sg[0:11, :], ps[:], ACTF.Sigmoid, bias=rb3[:], scale=1.0)
        nc.scalar.activation(ex10[:, sl], sg[0:11, :], ACTF.Exp, scale=10.0)

    # fetch AR1 result (ssz sum); compute tv = 7*e10 / max(||z||, eps)
    nc.sync.dma_start(arv[:, B:2 * B], cc1_out[:])
    for bs in range(NBS):
        sl = slice(bs * BSL, (bs + 1) * BSL)
        bc = ps_tile(128)
        nc.tensor.matmul(bc[:], lhsT=sel[:, 10, :],
                         rhs=ex10[:, sl], start=True, stop=True)
        nc.scalar.copy(arv[:, bs * BSL:(bs + 1) * BSL], bc[0:1, :])
    nc.vector.tensor_scalar_mul(arv[:, 0:B], arv[:, 0:B], 7.0)
    s_v = arv[:, B:2 * B]
    nc.scalar.sqrt(s_v, s_v)
    nc.vector.tensor_scalar_max(s_v, s_v, EPS_NORM)
    nc.vector.reciprocal(uv[:], s_v)
    nc.vector.tensor_tensor(out=tv[:], in0=arv[:, 0:B], in1=uv[:], op=ALU.mult)

    # ================= folds: router groups + g1, then stream rest =====
    for g in STREAM_GS[:3]:
        fold_group(router_raws[g], g)
    fold_group(raws1, g1)

    for g in STREAM_GS[4:]:
        xt, ws = stream_group(xT_d[g], w_d[g])
        bp_g = bpp.tile([G, DS], bf16, tag="bpg", name="bpg")
        nc.gpsimd.dma_start(bp_g[:], bp_d[:, g, :])
        raws = group_matmuls(xt, ws, bp_g)
        fold_group(raws, g)

    # ================= tail: P = A + t*z; q = ||P||^2; AR2; scale ======
    for bs in range(NBS):
        sl = slice(bs * BSL, (bs + 1) * BSL)
        btv = ps_tile(128)
        nc.tensor.matmul(btv[:], lhsT=ones1[:],
                         rhs=tv[:, sl], start=True, stop=True)
        for ci, (m0, msz) in enumerate(CHUNKS):
            t2 = ctp.tile([128, BSL], f32, tag="ctmp", name="ctmp")
            nc.vector.tensor_tensor(out=t2[0:msz, :], in0=z[ci][:, sl],
                                    in1=btv[0:msz, :], op=ALU.mult)
            nc.vector.tensor_tensor(out=A[ci][:, sl], in0=A[ci][:, sl],
                                    in1=t2[0:msz, :], op=ALU.add)
        psq = ps_tile(1)
        for ci, (m0, msz) in enumerate(CHUNKS):
            sq = b1k.tile([128, B], f32r, tag="big1k", name="big1k")
            nc.scalar.square(sq[0:msz, 0:BSL], A[ci][:, sl])
            nc.tensor.matmul(psq[:], lhsT=onesb[0:msz, :],
                             rhs=sq[0:msz, 0:BSL],
                             start=(ci == 0), stop=(ci == len(CHUNKS) - 1))
        nc.scalar.copy(arv[:, bs * BSL:(bs + 1) * BSL], psq[:])
    nc.sync.dma_start(cc2_in[:], arv[:, 0:B])
    nc.gpsimd.collective_compute(
        "AllReduce", ALU.add,
        ins=[cc2_in.opt()], outs=[cc2_out.opt()],
        replica_groups=[list(range(NCORES))])
    nc.sync.dma_start(arv[:, 0:B], cc2_out[:])

    q_v = arv[:, 0:B]
    nc.scalar.sqrt(q_v, q_v)
    nc.vector.tensor_scalar_max(q_v, q_v, EPS_NORM)
    nc.vector.reciprocal(uv[:], q_v)
    for bs in range(NBS):
        sl = slice(bs * BSL, (bs + 1) * BSL)
        bu = ps_tile(128)
        nc.tensor.matmul(bu[:], lhsT=ones1[:],
                         rhs=uv[:, sl], start=True, stop=True)
        for ci, (m0, msz) in enumerate(CHUNKS):
            nc.vector.tensor_tensor(out=outsb[ci][:, sl], in0=A[ci][:, sl],
                                    in1=bu[0:msz, :], op=ALU.mult)
    for ci, (m0, msz) in enumerate(CHUNKS):
        nc.sync.dma_start(outT_d[m0:m0 + msz, :], outsb[ci][:])

    for p in reversed(list(pools.values())):
        p.__exit__(None, None, None)


def _build_nc():
    nc = bacc.Bacc("TRN2", target_bir_lowering=False, debug=False,
                   num_devices=NCORES)
    with tile.TileContext(nc) as tc:
        with nc.allow_low_precision(reason="bf16 streams / f32r reductions are intentional"):
            _emit(nc, tc)
    nc.compile()
    return nc


def _as_bf16(a):
    return np.ascontiguousarray(a.astype(ml_dtypes.bfloat16))


def _host_prep(inputs):
    x_enc = np.asarray(inputs["x_enc"], dtype=np.float32)
    x_ib = np.asarray(inputs["x_ib"], dtype=np.float32)
    x_uni = np.asarray(inputs["x_uni"], dtype=np.float32)
    W_proj = np.asarray(inputs["W_proj"], dtype=np.float32)
    b_proj = np.asarray(inputs["b_proj"], dtype=np.float32)
    W_ib = np.asarray(inputs["W_ib"], dtype=np.float32)
    b_ib = np.asarray(inputs["b_ib"], dtype=np.float32)

    # x_enc [N,B,K] -> [N, 128, KT, B] partition-major bf16
    xT = _as_bf16(x_enc.transpose(0, 2, 1).reshape(N, KT, 128, B).transpose(0, 2, 1, 3))
    # x_ib [B,K] -> [128, KT, B]
    xibT = _as_bf16(x_ib.T.reshape(KT, 128, B).transpose(1, 0, 2))
    sel = np.zeros((11, 11, 128), dtype=np.float32)
    for q in range(11):
        sel[q, q, :] = 1.0
    rb3 = np.ascontiguousarray(np.asarray(inputs["r_b3"], np.float32).reshape(11, 1))
    # r_w1 [3072, 512] -> [3, 128, KT, 512]
    rw1 = _as_bf16(np.asarray(inputs["r_w1"], np.float32)
                   .reshape(3, KT, 128, BSL).transpose(0, 2, 1, 3))
    rw2 = _as_bf16(np.asarray(inputs["r_w2"], np.float32)
                   .reshape(4, 128, 100).transpose(1, 0, 2))
    rw3 = _as_bf16(np.asarray(inputs["r_w3"], np.float32))
    ones_host = np.ones((128, BSL), dtype=np.float32)

    in_maps = []
    for c in range(NCORES):
        ds = slice(c * DS, (c + 1) * DS)
        # W_proj [N,G,K,D] ds-slice -> [N, 128, KT, DS, G]
        wc = _as_bf16(W_proj[:, :, :, ds].reshape(N, G, KT, 128, DS)
                      .transpose(0, 3, 2, 4, 1))
        wibc = _as_bf16(W_ib[:, :, ds].reshape(G, KT, 128, DS)
                        .transpose(2, 1, 3, 0))
        in_maps.append({
            "xT": xT,
            "xibT": xibT,
            "w": wc,
            "wib": wibc,
            "xuT": _as_bf16(x_uni[:, ds].T),
            "bp": _as_bf16(b_proj[:, :, ds].transpose(1, 0, 2)),
            "bib": _as_bf16(b_ib[:, ds]),
            "rw1": rw1,
            "rw2": rw2,
            "rw3": rw3,
            "rb3": rb3,
            "sel": sel,
            "onesd": ones_host,
        })
    return in_maps


def kernel(**inputs):
    global LAST_RESULTS
    if "nc" not in _NC_CACHE:
        _NC_CACHE["nc"] = _build_nc()
    nc = _NC_CACHE["nc"]
    in_maps = _host_prep(inputs)
    res = run_bass_kernel_spmd(nc, in_maps, list(range(NCORES)))
    LAST_RESULTS = res
    full = np.concatenate([res.results[c]["outT"] for c in range(NCORES)], axis=0)
    return np.ascontiguousarray(full.T)
